# revision 1
# baseline (speedup 1.0000x reference)
"""Trainium2 Bass kernel for the quadtree-sum (CNNDST) problem.

Math: the reference recursion computes, for each sample b,
    out[b, j] = sum over (r, c) with (r AND c) == j of M[b, 0, r, c]
and it can be run IN-PLACE: after level k, the partial sum for entry e lives
at positions (r, c) with r === c === e (mod 2^k).  Every level is then two
tensor_tensor adds with a diagonal-strided access pattern, and the final
values land on the diagonal.

Layout (per core, 8 samples):
  - 2 samples per "group": each sample is split into 64 blocks of 128x128,
    one block per SBUF partition (2 samples -> 128 partitions).  Quadtree
    levels 1..7 never cross block boundaries -> in-place diag recursion,
    2 TT ops per level, 14 per group.
  - Level 7 writes its 64 g0 finals contiguously into D[:, 0:64] and a
    ScalarE copy extracts the 64 g1 finals (block diagonal m in [64,128))
    into D[:, 64:128].  D[64*si + q][e] = entry-e partial of block q.
  - Levels 8..10 pair *blocks*, i.e. reduce along partitions.  That's a
    single tiny TensorE matmul: out[(si,m2)][e] = sum_q W[q][(si,m2)] * D,
    with W[64si+8Bi+Bj][8si'+m2] = (si==si') * (Bi AND Bj == m2), a 0/1
    block-diagonal constant supplied as a second input.
  - PSUM -> SBUF copy on ScalarE, then one DMA writes out[2g+si][128*m2+e].
  - Load DMAs ride the SP (sync) HWDGE queue; everything post-compute rides
    the ACT (scalar) queue so load issue never stalls on compute waits.
"""

import numpy as np

import concourse.bass as bass
import concourse.tile as tile
from concourse import bacc
from concourse import mybir
import concourse.bass_utils as bass_utils
from concourse.ap import AP

F32 = mybir.dt.float32

SPC = 8          # samples per core
NCORES = 8
IMG = 1024
BLK = 128        # block side; 64 blocks/sample, one per partition
XPITCH = BLK * BLK   # 16384 elems per partition of X


def _mkap(tile_ap, p_off, p_cnt, off, dims):
    """AP over `tile_ap` (a [P, F] tile): partition window + free dims.

    dims: list of (step, count) free dims, outer first.
    """
    pitch = tile_ap.ap[0][0]
    return AP(
        tile_ap.tensor,
        tile_ap.offset + p_off * pitch + off,
        [[pitch, p_cnt]] + [[s, c] for (s, c) in dims],
    )


def _ctg(counts):
    """Contiguous dims for the given counts (outer first)."""
    dims = []
    step = 1
    for c in reversed(counts):
        dims.append((step, c))
        step *= c
    return list(reversed(dims))


def _emit_block_levels(nc, x, tmp, d):
    """Levels 1..7, block-local, in-place in x ([128, 16384])."""
    for k in range(1, 8):
        s = 1 << (k - 1)
        t_cnt, e_cnt, u_cnt = 64 // s, s, 64 // s
        adims = [(256 * s, t_cnt), (129, e_cnt), (2 * s, u_cnt)]
        a = _mkap(x, 0, 128, 0, adims)
        b = _mkap(x, 0, 128, s, adims)
        c = _mkap(x, 0, 128, 128 * s, adims)
        t_ap = _mkap(tmp, 0, 128, 0, _ctg([t_cnt, e_cnt, u_cnt]))
        nc.vector.tensor_add(t_ap, a, b)
        if k < 7:
            nc.vector.tensor_add(a, t_ap, c)
        else:
            # 64 g0 finals -> d[:, 0:64] (contiguous)
            dd = _mkap(d, 0, 128, 0, [(64, 1), (1, 64), (64, 1)])
            nc.vector.tensor_add(dd, t_ap, c)
            # 64 g1 finals: block diagonal m in [64, 128) -> d[:, 64:128]
            src = _mkap(x, 0, 128, 129 * 64, [(129, 64)])
            dst = _mkap(d, 0, 128, 64, [(1, 64)])
            nc.scalar.copy(dst, src)


def _emit_body(nc, tc, ctx, m, out, w_t, oall, pools):
    xpool, tpool, dpool, ppool = pools
    for g in range(4):
        x = xpool.tile([128, XPITCH], F32)
        # the host supplies m block-contiguous ([s][bi][bj][r][c]), so a
        # group's 2 samples load as one fully-contiguous 8MB DMA
        src = AP(m.ap().tensor, 2 * g * IMG * IMG, [[XPITCH, 128], [1, XPITCH]])
        dst = AP(x.tensor, x.offset, [[XPITCH, 128], [1, XPITCH]])
        nc.sync.dma_start(dst, src)

        tmp = tpool.tile([128, 4096], F32)
        d = dpool.tile([128, 128], F32)
        _emit_block_levels(nc, x, tmp, d)

        # levels 8..10: block-AND reduction as one matmul
        ps = ppool.tile([16, 128], F32)
        nc.tensor.matmul(ps[:], w_t[:], d[:], start=True, stop=True)
        # stage results in SBUF; DRAM writes happen once at the end
        nc.scalar.copy(_mkap(oall, 0, 16, g * 128, [(1, 128)]), ps[:])

    # two end-of-kernel DMAs: out[2g+si][128*m2 + e] = oall[8si+m2][128g + e]
    for si in range(2):
        osrc = _mkap(oall, 8 * si, 8, 0, [(128, 4), (1, 128)])
        odst = AP(out.ap().tensor, si * 1024, [[128, 8], [2048, 4], [1, 128]])
        nc.scalar.dma_start(odst, osrc)


def make_w() -> np.ndarray:
    w = np.zeros((128, 16), np.float32)
    for si in range(2):
        for bi in range(8):
            for bj in range(8):
                w[64 * si + 8 * bi + bj][8 * si + (bi & bj)] = 1.0
    return w


def build(iters: int = 1) -> bass.Bass:
    nc = bacc.Bacc("TRN2", target_bir_lowering=False, debug=False)
    m = nc.dram_tensor("m", [SPC, 64 * XPITCH], F32, kind="ExternalInput")
    w = nc.dram_tensor("w", [128, 16], F32, kind="ExternalInput")
    out = nc.dram_tensor("out", [SPC, 1024], F32, kind="ExternalOutput")
    from contextlib import ExitStack

    with tile.TileContext(nc) as tc:
        with ExitStack() as ctx:
            xpool = ctx.enter_context(tc.tile_pool(name="x", bufs=2))
            tpool = ctx.enter_context(tc.tile_pool(name="tmp", bufs=1))
            dpool = ctx.enter_context(tc.tile_pool(name="d", bufs=2))
            ppool = ctx.enter_context(tc.tile_pool(name="ps", bufs=2, space="PSUM"))
            wpool = ctx.enter_context(tc.tile_pool(name="w", bufs=1))
            w_t = wpool.tile([128, 16], F32)
            oall = wpool.tile([16, 512], F32, tag="oall")
            nc.sync.dma_start(w_t[:], w.ap())
            pools = (xpool, tpool, dpool, ppool)
            if iters == 1:
                _emit_body(nc, tc, ctx, m, out, w_t, oall, pools)
            else:
                with tc.For_i(0, iters, 1):
                    _emit_body(nc, tc, ctx, m, out, w_t, oall, pools)
    nc.compile()
    return nc


def kernel(**inputs) -> np.ndarray:
    M = np.asarray(inputs["M"], dtype=np.float32)
    B = M.shape[0]
    # block-contiguous relayout: [s][bi][bj][r][c]
    M3 = np.ascontiguousarray(
        M.reshape(B, 8, BLK, 8, BLK).transpose(0, 1, 3, 2, 4)
    ).reshape(B, 64 * XPITCH)
    nc = build(1)
    w = make_w()
    in_maps = [
        {"m": M3[i * SPC:(i + 1) * SPC], "w": w} for i in range(NCORES)
    ]
    res = bass_utils.run_bass_kernel_spmd(nc, in_maps, core_ids=list(range(NCORES)))
    out = np.concatenate([r["out"] for r in res.results], axis=0)  # [B, 1024]
    return out.reshape(B, 1024, 1, 1, 1)



# revision 2
# speedup vs baseline: 16.3896x; 16.3896x over previous
"""Trainium2 Bass kernel v4 for the quadtree-sum (CNNDST) problem.

Same math/layout as v2/v3.  Changes vs v3:
  - two-op in-place accumulation (dst = a+b; dst += c) - no tmp buffer,
    no tmp WAR serialization
  - DVE:Pool work split ~2:1 (cost model has Pool at ~half DVE rate)
  - level 1 is split across engines within each prefix block, so the
    tail-critical j9=1 block (awaits the last load chunk) is short
"""

import numpy as np

import concourse.bass as bass
import concourse.tile as tile
from concourse import bacc
from concourse import mybir
import concourse.bass_utils as bass_utils
from concourse.ap import AP

F32 = mybir.dt.float32

SPC = 8
NCORES = 8
XPITCH = 16384
Q = 4096


def _mkap(tile_ap, p_off, p_cnt, off, dims):
    pitch = tile_ap.ap[0][0]
    return AP(
        tile_ap.tensor,
        tile_ap.offset + p_off * pitch + off,
        [[pitch, p_cnt]] + [[s, c] for (s, c) in dims],
    )


def _acc3(eng, dst, a, b, c):
    eng.tensor_add(dst, a, b)
    eng.tensor_add(dst, dst, c)


def _emit_levels(nc, x, bufB, bufA2, d):
    # ---- level 0: dst bufB[0:4096]; f-split DVE 2688 / Pool 1408 ----
    for f0, fn, eng in ((0, 2688, nc.vector), (2688, 1408, nc.gpsimd)):
        _acc3(eng,
              _mkap(bufB, 0, 128, f0, [(1, fn)]),
              _mkap(x, 0, 128, f0, [(1, fn)]),
              _mkap(x, 0, 128, Q + f0, [(1, fn)]),
              _mkap(x, 0, 128, 2 * Q + f0, [(1, fn)]))

    # ---- level 1: blocks b0 (src bufB) and b1 (src x q11); both engine-split ----
    R = 1024
    for blk, (src, base) in enumerate(((bufB, 0), (x, 3 * Q))):
        for f0, fn, eng in ((0, 672, nc.vector), (672, 352, nc.gpsimd)):
            _acc3(eng,
                  _mkap(bufA2, 0, 128, 2 * R * blk + f0, [(1, fn)]),
                  _mkap(src, 0, 128, base + f0, [(1, fn)]),
                  _mkap(src, 0, 128, base + R + f0, [(1, fn)]),
                  _mkap(src, 0, 128, base + 2 * R + f0, [(1, fn)]))
        nc.scalar.copy(_mkap(bufA2, 0, 128, 2 * R * blk + R, [(1, R)]),
                       _mkap(src, 0, 128, base + 3 * R, [(1, R)]))

    # ---- levels 2..6: block-range split DVE/Pool ----
    cur = bufA2
    dve_blocks = {2: 3, 3: 5, 4: 10, 5: 21, 6: 42}
    for i in range(2, 7):
        R = 1 << (12 - 2 * i)
        P = 1 << i
        nxt = d if i == 6 else (bufB if i % 2 == 0 else bufA2)
        bd = dve_blocks[i]
        for b0, nb, eng in ((0, bd, nc.vector), (bd, P - bd, nc.gpsimd)):
            sdims = [(4 * R, nb), (1, R)] if R > 1 else [(4, nb)]
            ddims = [(2 * R, nb), (1, R)] if R > 1 else [(2, nb)]
            _acc3(eng,
                  _mkap(nxt, 0, 128, b0 * 2 * R, ddims),
                  _mkap(cur, 0, 128, b0 * 4 * R, sdims),
                  _mkap(cur, 0, 128, b0 * 4 * R + R, sdims),
                  _mkap(cur, 0, 128, b0 * 4 * R + 2 * R, sdims))
        cdims = [(4 * R, P), (1, R)] if R > 1 else [(4, P)]
        odims = [(2 * R, P), (1, R)] if R > 1 else [(2, P)]
        nc.scalar.copy(_mkap(nxt, 0, 128, R, odims),
                       _mkap(cur, 0, 128, 3 * R, cdims))
        cur = nxt


def _emit_body(nc, m, w, w_t, oall, pools):
    xpool, bpool, dpool, ppool = pools
    for g in range(4):
        x = xpool.tile([128, XPITCH], F32)
        for c in range(4):
            src = AP(m.ap().tensor, g * 128 * XPITCH + c * Q,
                     [[XPITCH, 128], [1, Q]])
            dst = AP(x.tensor, x.offset + c * Q, [[x.ap[0][0], 128], [1, Q]])
            nc.sync.dma_start(dst, src)
        if g == 0:
            nc.sync.dma_start(w_t[:], w.ap())

        bufB = bpool.tile([128, 4096], F32, tag="bufB")
        bufA2 = bpool.tile([128, 4096], F32, tag="bufA2")
        d = dpool.tile([128, 128], F32)
        _emit_levels(nc, x, bufB, bufA2, d)

        ps = ppool.tile([16, 128], F32)
        nc.tensor.matmul(ps[:], w_t[:], d[:], start=True, stop=True)
        nc.scalar.copy(_mkap(oall, 0, 16, g * 128, [(1, 128)]), ps[:])


def make_w() -> np.ndarray:
    w = np.zeros((128, 16), np.float32)
    for p in range(128):
        si = p >> 6
        r0, c0 = (p >> 5) & 1, (p >> 4) & 1
        r1, c1 = (p >> 3) & 1, (p >> 2) & 1
        r2, c2 = (p >> 1) & 1, p & 1
        mm = (r0 & c0) | ((r1 & c1) << 1) | ((r2 & c2) << 2)
        w[p][8 * si + mm] = 1.0
    return w


def build(iters: int = 1) -> bass.Bass:
    nc = bacc.Bacc("TRN2", target_bir_lowering=False, debug=False)
    m = nc.dram_tensor("m", [4, 128 * XPITCH], F32, kind="ExternalInput")
    w = nc.dram_tensor("w", [128, 16], F32, kind="ExternalInput")
    out = nc.dram_tensor("out", [16, 512], F32, kind="ExternalOutput")
    from contextlib import ExitStack

    with tile.TileContext(nc) as tc:
        with ExitStack() as ctx:
            xpool = ctx.enter_context(tc.tile_pool(name="x", bufs=2))
            bpool = ctx.enter_context(tc.tile_pool(name="b", bufs=1))
            dpool = ctx.enter_context(tc.tile_pool(name="d", bufs=2))
            ppool = ctx.enter_context(tc.tile_pool(name="ps", bufs=2, space="PSUM"))
            wpool = ctx.enter_context(tc.tile_pool(name="w", bufs=1))
            w_t = wpool.tile([128, 16], F32)
            oall = wpool.tile([16, 512], F32, tag="oall")
            pools = (xpool, bpool, dpool, ppool)
            if iters == 1:
                _emit_body(nc, m, w, w_t, oall, pools)
            else:
                with tc.For_i(0, iters, 1):
                    _emit_body(nc, m, w, w_t, oall, pools)
            nc.sync.dma_start(out.ap(), oall[:])
    nc.compile()
    return nc


def _relayout(M: np.ndarray) -> np.ndarray:
    B = M.shape[0]
    Mv = M.reshape((B,) + (2,) * 20)
    r = {k: 1 + (9 - k) for k in range(10)}
    c = {k: 11 + (9 - k) for k in range(10)}
    perm = [0,
            r[0], c[0], r[1], c[1], r[2], c[2],
            r[9], c[9], r[8], c[8], r[7], c[7], r[6], c[6],
            r[5], c[5], r[4], c[4], r[3], c[3]]
    return np.ascontiguousarray(Mv.transpose(perm)).reshape(B, 64, XPITCH)


def _unscramble(res_cores: list[np.ndarray]) -> np.ndarray:
    outs = []
    for rk in res_cores:
        v = rk.reshape(2, 8, 4, 128)          # [si][m][g][e]
        v = v.transpose(2, 0, 3, 1)           # [g][si][e][m]
        outs.append(v.reshape(8, 1024))
    return np.concatenate(outs, axis=0)


def kernel(**inputs) -> np.ndarray:
    M = np.asarray(inputs["M"], dtype=np.float32)
    B = M.shape[0]
    X = _relayout(M.reshape(B, 1024, 1024))
    X = X.reshape(B // 2, 2 * 64 * XPITCH)
    nc = build(1)
    w = make_w()
    in_maps = [
        {"m": X[4 * k:4 * k + 4].reshape(4, 128 * XPITCH), "w": w}
        for k in range(NCORES)
    ]
    res = bass_utils.run_bass_kernel_spmd(nc, in_maps, core_ids=list(range(NCORES)))
    out = _unscramble([r["out"] for r in res.results])
    return out.reshape(B, 1024, 1, 1, 1)


# revision 5
# speedup vs baseline: 28.0243x; 1.7099x over previous
"""Trainium2 Bass kernel v4 for the quadtree-sum (CNNDST) problem.

Same math/layout as v2/v3.  Changes vs v3:
  - two-op in-place accumulation (dst = a+b; dst += c) - no tmp buffer,
    no tmp WAR serialization
  - DVE:Pool work split ~2:1 (cost model has Pool at ~half DVE rate)
  - level 1 is split across engines within each prefix block, so the
    tail-critical j9=1 block (awaits the last load chunk) is short
"""

import numpy as np

import concourse.bass as bass
import concourse.tile as tile
from concourse import bacc
from concourse import mybir
import concourse.bass_utils as bass_utils
from concourse.ap import AP

F32 = mybir.dt.float32

SPC = 8
NCORES = 8
XPITCH = 16384
Q = 4096


def _mkap(tile_ap, p_off, p_cnt, off, dims):
    pitch = tile_ap.ap[0][0]
    return AP(
        tile_ap.tensor,
        tile_ap.offset + p_off * pitch + off,
        [[pitch, p_cnt]] + [[s, c] for (s, c) in dims],
    )


def _acc3(eng, dst, a, b, c):
    eng.tensor_add(dst, a, b)
    eng.tensor_add(dst, dst, c)


def _emit_levels(nc, x, bufB, bufA2, d):
    # ---- level 0: dst bufB[0:4096]; f-split DVE 2688 / Pool 1408 ----
    for f0, fn, eng in ((0, 2688, nc.vector), (2688, 1408, nc.gpsimd)):
        _acc3(eng,
              _mkap(bufB, 0, 128, f0, [(1, fn)]),
              _mkap(x, 0, 128, f0, [(1, fn)]),
              _mkap(x, 0, 128, Q + f0, [(1, fn)]),
              _mkap(x, 0, 128, 2 * Q + f0, [(1, fn)]))

    # ---- level 1: blocks b0 (src bufB) and b1 (src x q11); both engine-split ----
    R = 1024
    for blk, (src, base) in enumerate(((bufB, 0), (x, 3 * Q))):
        for f0, fn, eng in ((0, 672, nc.vector), (672, 352, nc.gpsimd)):
            _acc3(eng,
                  _mkap(bufA2, 0, 128, 2 * R * blk + f0, [(1, fn)]),
                  _mkap(src, 0, 128, base + f0, [(1, fn)]),
                  _mkap(src, 0, 128, base + R + f0, [(1, fn)]),
                  _mkap(src, 0, 128, base + 2 * R + f0, [(1, fn)]))
        nc.scalar.copy(_mkap(bufA2, 0, 128, 2 * R * blk + R, [(1, R)]),
                       _mkap(src, 0, 128, base + 3 * R, [(1, R)]))

    # ---- levels 2..6: block-range split DVE/Pool ----
    cur = bufA2
    dve_blocks = {2: 3, 3: 5, 4: 10, 5: 21, 6: 42}
    for i in range(2, 7):
        R = 1 << (12 - 2 * i)
        P = 1 << i
        nxt = d if i == 6 else (bufB if i % 2 == 0 else bufA2)
        bd = dve_blocks[i]
        for b0, nb, eng in ((0, bd, nc.vector), (bd, P - bd, nc.gpsimd)):
            sdims = [(4 * R, nb), (1, R)] if R > 1 else [(4, nb)]
            ddims = [(2 * R, nb), (1, R)] if R > 1 else [(2, nb)]
            _acc3(eng,
                  _mkap(nxt, 0, 128, b0 * 2 * R, ddims),
                  _mkap(cur, 0, 128, b0 * 4 * R, sdims),
                  _mkap(cur, 0, 128, b0 * 4 * R + R, sdims),
                  _mkap(cur, 0, 128, b0 * 4 * R + 2 * R, sdims))
        cdims = [(4 * R, P), (1, R)] if R > 1 else [(4, P)]
        odims = [(2 * R, P), (1, R)] if R > 1 else [(2, P)]
        nc.scalar.copy(_mkap(nxt, 0, 128, R, odims),
                       _mkap(cur, 0, 128, 3 * R, cdims))
        cur = nxt


def _emit_body(nc, m, w, w_t, oall, pools):
    xpool, bpool, dpool, ppool = pools
    for g in range(4):
        x = xpool.tile([128, XPITCH], F32)
        for c in range(4):
            src = AP(m.ap().tensor, g * 128 * XPITCH + c * Q,
                     [[XPITCH, 128], [1, Q]])
            dst = AP(x.tensor, x.offset + c * Q, [[x.ap[0][0], 128], [1, Q]])
            nc.sync.dma_start(dst, src)
        if g == 0:
            nc.sync.dma_start(w_t[:], w.ap())

        bufB = bpool.tile([128, 4096], F32, tag="bufB")
        bufA2 = bpool.tile([128, 4096], F32, tag="bufA2")
        d = dpool.tile([128, 128], F32)
        _emit_levels(nc, x, bufB, bufA2, d)

        ps = ppool.tile([16, 128], F32)
        nc.tensor.matmul(ps[:], w_t[:], d[:], start=True, stop=True)
        nc.scalar.copy(_mkap(oall, 0, 16, g * 128, [(1, 128)]), ps[:])


def make_w() -> np.ndarray:
    w = np.zeros((128, 16), np.float32)
    for p in range(128):
        si = p >> 6
        r0, c0 = (p >> 5) & 1, (p >> 4) & 1
        r1, c1 = (p >> 3) & 1, (p >> 2) & 1
        r2, c2 = (p >> 1) & 1, p & 1
        mm = (r0 & c0) | ((r1 & c1) << 1) | ((r2 & c2) << 2)
        w[p][8 * si + mm] = 1.0
    return w


def build(iters: int = 1, timing_internal: bool = False) -> bass.Bass:
    """timing_internal: 'm' is Internal DRAM (no host upload), zero-filled
    on device before the loop - for loop-differencing timing only."""
    nc = bacc.Bacc("TRN2", target_bir_lowering=False, debug=False)
    mkind = "Internal" if timing_internal else "ExternalInput"
    m = nc.dram_tensor("m", [4, 128 * XPITCH], F32, kind=mkind)
    w = nc.dram_tensor("w", [128, 16], F32, kind="ExternalInput")
    out = nc.dram_tensor("out", [16, 512], F32, kind="ExternalOutput")
    from contextlib import ExitStack

    with tile.TileContext(nc) as tc:
        with ExitStack() as ctx:
            xpool = ctx.enter_context(tc.tile_pool(name="x", bufs=2))
            bpool = ctx.enter_context(tc.tile_pool(name="b", bufs=1))
            dpool = ctx.enter_context(tc.tile_pool(name="d", bufs=2))
            ppool = ctx.enter_context(tc.tile_pool(name="ps", bufs=2, space="PSUM"))
            wpool = ctx.enter_context(tc.tile_pool(name="w", bufs=1))
            w_t = wpool.tile([128, 16], F32)
            oall = wpool.tile([16, 512], F32, tag="oall")
            pools = (xpool, bpool, dpool, ppool)
            if timing_internal:
                zpool = ctx.enter_context(tc.tile_pool(name="z", bufs=1))
                z = zpool.tile([128, 2048], F32)
                nc.vector.memset(z[:], 0.0)
                for ch in range(32):
                    nc.sync.dma_start(
                        AP(m.ap().tensor,
                           (ch // 8) * 128 * XPITCH + (ch % 8) * 2048,
                           [[XPITCH, 128], [1, 2048]]),
                        AP(z.tensor, z.offset, [[z.ap[0][0], 128], [1, 2048]]))
            if iters == 1:
                _emit_body(nc, m, w, w_t, oall, pools)
            else:
                with tc.For_i(0, iters, 1):
                    _emit_body(nc, m, w, w_t, oall, pools)
            nc.sync.dma_start(out.ap(), oall[:])
    nc.compile()
    return nc


def _relayout(M: np.ndarray) -> np.ndarray:
    B = M.shape[0]
    Mv = M.reshape((B,) + (2,) * 20)
    r = {k: 1 + (9 - k) for k in range(10)}
    c = {k: 11 + (9 - k) for k in range(10)}
    perm = [0,
            r[0], c[0], r[1], c[1], r[2], c[2],
            r[9], c[9], r[8], c[8], r[7], c[7], r[6], c[6],
            r[5], c[5], r[4], c[4], r[3], c[3]]
    return np.ascontiguousarray(Mv.transpose(perm)).reshape(B, 64, XPITCH)


def _unscramble(res_cores: list[np.ndarray]) -> np.ndarray:
    outs = []
    for rk in res_cores:
        v = rk.reshape(2, 8, 4, 128)          # [si][m][g][e]
        v = v.transpose(2, 0, 3, 1)           # [g][si][e][m]
        outs.append(v.reshape(8, 1024))
    return np.concatenate(outs, axis=0)


def kernel(**inputs) -> np.ndarray:
    M = np.asarray(inputs["M"], dtype=np.float32)
    B = M.shape[0]
    X = _relayout(M.reshape(B, 1024, 1024))
    X = X.reshape(B // 2, 2 * 64 * XPITCH)
    nc = build(1)
    w = make_w()
    in_maps = [
        {"m": X[4 * k:4 * k + 4].reshape(4, 128 * XPITCH), "w": w}
        for k in range(NCORES)
    ]
    res = bass_utils.run_bass_kernel_spmd(nc, in_maps, core_ids=list(range(NCORES)))
    out = _unscramble([r["out"] for r in res.results])
    return out.reshape(B, 1024, 1, 1, 1)


# revision 6
# speedup vs baseline: 28.2693x; 1.0087x over previous
"""Trainium2 Bass kernel for the quadtree-sum (CNNDST) problem.

Math: the reference quadtree recursion computes, for each sample b,
    out[b, j] = sum over (r, c) with (r AND c) == j of M[b, 0, r, c]
(j, r, c are 10-bit).  The per-bit AND factorizes, so the ten bit-pairs
(r_k, c_k) can be contracted in any order; contracting pair k maps
A[r_k=0,c_k=0] + A[0,1] + A[1,0] -> B[j_k=0] and A[1,1] -> B[j_k=1].

Layout per core (8 samples, data parallel over 8 cores; 4 groups of 2
samples per core).  Group tile [128 part x 16384 f32]:
    partition p = 64*si + (r0 c0 r1 c1 r2 c2)       (si = sample in pair)
    free f (MSB->LSB) = (r9 c9 r8 c8 ... r3 c3)     (host-side relayout)
Seven in-partition levels contract (r9,c9) .. (r3,c3); with the pair in
the two MSBs of the active array every level is 2 contiguous/2-dim-AP
adds (dst = q00+q01; dst += q10) plus a ScalarE copy of q11, ping-pong
compacting 16384 -> 128 elems/partition.  Design points:
  - in-place two-op accumulation: no tmp buffer, no tmp WAR stalls
  - adds split ~2:1 across DVE and Pool so both engines work; copies on
    ScalarE; all access patterns contiguous or 2-dim with long runs
  - level 0 writes no g1 copy: level 1's j9=1 block reads x's q11
    quarter in place
  - each 8MB group load is 4 f-chunks so level-0 compute starts after
    chunk 1; all DMAs ride qSP (single queue sustains ~390 GB/s; qACT /
    qPool big DMAs proved pathological on this setup)
Cross-partition levels (pairs 0..2) are one tiny PE matmul per group
with constant 0/1 W[128,16]: S[8*si+m][e] = sum_p W[p][8si+m] D[p][e],
where m = j&7, e = j>>3.  PSUM -> SBUF staging (ScalarE), one 32KB
output DMA at kernel end; host does the final [16,512] -> [8,1024]
index unscramble (free).
"""

import numpy as np

import concourse.bass as bass
import concourse.tile as tile
from concourse import bacc
from concourse import mybir
import concourse.bass_utils as bass_utils
from concourse.ap import AP

F32 = mybir.dt.float32

SPC = 8
NCORES = 8
XPITCH = 16384
Q = 4096


def _mkap(tile_ap, p_off, p_cnt, off, dims):
    pitch = tile_ap.ap[0][0]
    return AP(
        tile_ap.tensor,
        tile_ap.offset + p_off * pitch + off,
        [[pitch, p_cnt]] + [[s, c] for (s, c) in dims],
    )


def _acc3(eng, dst, a, b, c):
    eng.tensor_add(dst, a, b)
    eng.tensor_add(dst, dst, c)


def _emit_levels(nc, x, bufB, bufA2, d):
    # ---- level 0: dst bufB[0:4096]; f-split DVE 2688 / Pool 1408 ----
    for f0, fn, eng in ((0, 2688, nc.vector), (2688, 1408, nc.gpsimd)):
        _acc3(eng,
              _mkap(bufB, 0, 128, f0, [(1, fn)]),
              _mkap(x, 0, 128, f0, [(1, fn)]),
              _mkap(x, 0, 128, Q + f0, [(1, fn)]),
              _mkap(x, 0, 128, 2 * Q + f0, [(1, fn)]))

    # ---- level 1: blocks b0 (src bufB) and b1 (src x q11); both engine-split ----
    R = 1024
    for blk, (src, base) in enumerate(((bufB, 0), (x, 3 * Q))):
        for f0, fn, eng in ((0, 672, nc.vector), (672, 352, nc.gpsimd)):
            _acc3(eng,
                  _mkap(bufA2, 0, 128, 2 * R * blk + f0, [(1, fn)]),
                  _mkap(src, 0, 128, base + f0, [(1, fn)]),
                  _mkap(src, 0, 128, base + R + f0, [(1, fn)]),
                  _mkap(src, 0, 128, base + 2 * R + f0, [(1, fn)]))
        nc.scalar.copy(_mkap(bufA2, 0, 128, 2 * R * blk + R, [(1, R)]),
                       _mkap(src, 0, 128, base + 3 * R, [(1, R)]))

    # ---- levels 2..6: block-range split DVE/Pool ----
    cur = bufA2
    dve_blocks = {2: 3, 3: 5, 4: 10, 5: 21, 6: 42}
    for i in range(2, 7):
        R = 1 << (12 - 2 * i)
        P = 1 << i
        nxt = d if i == 6 else (bufB if i % 2 == 0 else bufA2)
        bd = dve_blocks[i]
        for b0, nb, eng in ((0, bd, nc.vector), (bd, P - bd, nc.gpsimd)):
            sdims = [(4 * R, nb), (1, R)] if R > 1 else [(4, nb)]
            ddims = [(2 * R, nb), (1, R)] if R > 1 else [(2, nb)]
            _acc3(eng,
                  _mkap(nxt, 0, 128, b0 * 2 * R, ddims),
                  _mkap(cur, 0, 128, b0 * 4 * R, sdims),
                  _mkap(cur, 0, 128, b0 * 4 * R + R, sdims),
                  _mkap(cur, 0, 128, b0 * 4 * R + 2 * R, sdims))
        cdims = [(4 * R, P), (1, R)] if R > 1 else [(4, P)]
        odims = [(2 * R, P), (1, R)] if R > 1 else [(2, P)]
        nc.scalar.copy(_mkap(nxt, 0, 128, R, odims),
                       _mkap(cur, 0, 128, 3 * R, cdims))
        cur = nxt


def _emit_body(nc, m, w, w_t, oall, pools):
    xpool, bpool, dpool, ppool = pools
    for g in range(4):
        x = xpool.tile([128, XPITCH], F32)
        for c in range(4):
            src = AP(m.ap().tensor, g * 128 * XPITCH + c * Q,
                     [[XPITCH, 128], [1, Q]])
            dst = AP(x.tensor, x.offset + c * Q, [[x.ap[0][0], 128], [1, Q]])
            nc.sync.dma_start(dst, src)
        if g == 0:
            nc.sync.dma_start(w_t[:], w.ap())

        bufB = bpool.tile([128, 4096], F32, tag="bufB")
        bufA2 = bpool.tile([128, 4096], F32, tag="bufA2")
        d = dpool.tile([128, 128], F32)
        _emit_levels(nc, x, bufB, bufA2, d)

        ps = ppool.tile([16, 128], F32)
        nc.tensor.matmul(ps[:], w_t[:], d[:], start=True, stop=True)
        nc.scalar.copy(_mkap(oall, 0, 16, g * 128, [(1, 128)]), ps[:])


def make_w() -> np.ndarray:
    w = np.zeros((128, 16), np.float32)
    for p in range(128):
        si = p >> 6
        r0, c0 = (p >> 5) & 1, (p >> 4) & 1
        r1, c1 = (p >> 3) & 1, (p >> 2) & 1
        r2, c2 = (p >> 1) & 1, p & 1
        mm = (r0 & c0) | ((r1 & c1) << 1) | ((r2 & c2) << 2)
        w[p][8 * si + mm] = 1.0
    return w


def build(iters: int = 1, timing_internal: bool = False) -> bass.Bass:
    """timing_internal: 'm' is Internal DRAM (no host upload), zero-filled
    on device before the loop - for loop-differencing timing only."""
    nc = bacc.Bacc("TRN2", target_bir_lowering=False, debug=False)
    mkind = "Internal" if timing_internal else "ExternalInput"
    m = nc.dram_tensor("m", [4, 128 * XPITCH], F32, kind=mkind)
    w = nc.dram_tensor("w", [128, 16], F32, kind="ExternalInput")
    out = nc.dram_tensor("out", [16, 512], F32, kind="ExternalOutput")
    from contextlib import ExitStack

    with tile.TileContext(nc) as tc:
        with ExitStack() as ctx:
            xpool = ctx.enter_context(tc.tile_pool(name="x", bufs=2))
            bpool = ctx.enter_context(tc.tile_pool(name="b", bufs=1))
            dpool = ctx.enter_context(tc.tile_pool(name="d", bufs=2))
            ppool = ctx.enter_context(tc.tile_pool(name="ps", bufs=2, space="PSUM"))
            wpool = ctx.enter_context(tc.tile_pool(name="w", bufs=1))
            w_t = wpool.tile([128, 16], F32)
            oall = wpool.tile([16, 512], F32, tag="oall")
            pools = (xpool, bpool, dpool, ppool)
            if timing_internal:
                zpool = ctx.enter_context(tc.tile_pool(name="z", bufs=1))
                z = zpool.tile([128, 2048], F32)
                nc.vector.memset(z[:], 0.0)
                for ch in range(32):
                    nc.sync.dma_start(
                        AP(m.ap().tensor,
                           (ch // 8) * 128 * XPITCH + (ch % 8) * 2048,
                           [[XPITCH, 128], [1, 2048]]),
                        AP(z.tensor, z.offset, [[z.ap[0][0], 128], [1, 2048]]))
            if iters == 1:
                _emit_body(nc, m, w, w_t, oall, pools)
            else:
                with tc.For_i(0, iters, 1):
                    _emit_body(nc, m, w, w_t, oall, pools)
            nc.sync.dma_start(out.ap(), oall[:])
    nc.compile()
    return nc


def _relayout(M: np.ndarray) -> np.ndarray:
    B = M.shape[0]
    Mv = M.reshape((B,) + (2,) * 20)
    r = {k: 1 + (9 - k) for k in range(10)}
    c = {k: 11 + (9 - k) for k in range(10)}
    perm = [0,
            r[0], c[0], r[1], c[1], r[2], c[2],
            r[9], c[9], r[8], c[8], r[7], c[7], r[6], c[6],
            r[5], c[5], r[4], c[4], r[3], c[3]]
    return np.ascontiguousarray(Mv.transpose(perm)).reshape(B, 64, XPITCH)


def _unscramble(res_cores: list[np.ndarray]) -> np.ndarray:
    outs = []
    for rk in res_cores:
        v = rk.reshape(2, 8, 4, 128)          # [si][m][g][e]
        v = v.transpose(2, 0, 3, 1)           # [g][si][e][m]
        outs.append(v.reshape(8, 1024))
    return np.concatenate(outs, axis=0)


def kernel(**inputs) -> np.ndarray:
    M = np.asarray(inputs["M"], dtype=np.float32)
    B = M.shape[0]
    X = _relayout(M.reshape(B, 1024, 1024))
    X = X.reshape(B // 2, 2 * 64 * XPITCH)
    nc = build(1)
    w = make_w()
    in_maps = [
        {"m": X[4 * k:4 * k + 4].reshape(4, 128 * XPITCH), "w": w}
        for k in range(NCORES)
    ]
    res = bass_utils.run_bass_kernel_spmd(nc, in_maps, core_ids=list(range(NCORES)))
    out = _unscramble([r["out"] for r in res.results])
    return out.reshape(B, 1024, 1, 1, 1)


# revision 7
# speedup vs baseline: 31.9832x; 1.1314x over previous
"""Trainium2 Bass kernel for the quadtree-sum (CNNDST) problem.

Math: the reference quadtree recursion computes, for each sample b,
    out[b, j] = sum over (r, c) with (r AND c) == j of M[b, 0, r, c]
(j, r, c are 10-bit).  The per-bit AND factorizes, so the ten bit-pairs
(r_k, c_k) can be contracted in any order; contracting pair k maps
A[r_k=0,c_k=0] + A[0,1] + A[1,0] -> B[j_k=0] and A[1,1] -> B[j_k=1].

Layout per core (8 samples, data parallel over 8 cores; 4 groups of 2
samples per core).  Group tile [128 part x 16384 f32]:
    partition p = 64*si + (r0 c0 r1 c1 r2 c2)       (si = sample in pair)
    free f (MSB->LSB) = (r9 c9 r8 c8 ... r3 c3)     (host-side relayout)
Seven in-partition levels contract (r9,c9) .. (r3,c3); with the pair in
the two MSBs of the active array every level is 2 contiguous/2-dim-AP
adds (dst = q00+q01; dst += q10) plus a ScalarE copy of q11, ping-pong
compacting 16384 -> 128 elems/partition.  Design points:
  - in-place two-op accumulation: no tmp buffer, no tmp WAR stalls
  - all adds on DVE (measured faster than a DVE+Pool split; Pool is ~2x
    slower and cross-engine sync eats the gain); copies on ScalarE; all
    access patterns contiguous or 2-dim with long runs
  - level 0 writes no g1 copy: level 1's j9=1 block reads x's q11
    quarter in place
  - each 8MB group load is 4 f-chunks so level-0 compute starts after
    chunk 1; all DMAs ride qSP (single queue sustains ~390 GB/s; qACT /
    qPool big DMAs proved pathological on this setup)
Cross-partition levels (pairs 0..2) are one tiny PE matmul per group
with constant 0/1 W[128,16]: S[8*si+m][e] = sum_p W[p][8si+m] D[p][e],
where m = j&7, e = j>>3.  PSUM -> SBUF staging (ScalarE), one 32KB
output DMA at kernel end; host does the final [16,512] -> [8,1024]
index unscramble (free).
"""

import numpy as np

import concourse.bass as bass
import concourse.tile as tile
from concourse import bacc
from concourse import mybir
import concourse.bass_utils as bass_utils
from concourse.ap import AP

F32 = mybir.dt.float32

SPC = 8
NCORES = 8
XPITCH = 16384
Q = 4096


def _mkap(tile_ap, p_off, p_cnt, off, dims):
    pitch = tile_ap.ap[0][0]
    return AP(
        tile_ap.tensor,
        tile_ap.offset + p_off * pitch + off,
        [[pitch, p_cnt]] + [[s, c] for (s, c) in dims],
    )


def _acc3(eng, dst, a, b, c):
    eng.tensor_add(dst, a, b)
    eng.tensor_add(dst, dst, c)


def _emit_levels(nc, x, bufB, bufA2, d):
    # ---- level 0: dst bufB[0:4096]; all adds on DVE (measured: DVE-only
    # beats a DVE+Pool split - cross-engine sync costs more than Pool adds) ----
    for f0, fn, eng in ((0, 4096, nc.vector),):
        _acc3(eng,
              _mkap(bufB, 0, 128, f0, [(1, fn)]),
              _mkap(x, 0, 128, f0, [(1, fn)]),
              _mkap(x, 0, 128, Q + f0, [(1, fn)]),
              _mkap(x, 0, 128, 2 * Q + f0, [(1, fn)]))

    # ---- level 1: blocks b0 (src bufB) and b1 (src x q11); both engine-split ----
    R = 1024
    for blk, (src, base) in enumerate(((bufB, 0), (x, 3 * Q))):
        for f0, fn, eng in ((0, 1024, nc.vector),):
            _acc3(eng,
                  _mkap(bufA2, 0, 128, 2 * R * blk + f0, [(1, fn)]),
                  _mkap(src, 0, 128, base + f0, [(1, fn)]),
                  _mkap(src, 0, 128, base + R + f0, [(1, fn)]),
                  _mkap(src, 0, 128, base + 2 * R + f0, [(1, fn)]))
        nc.scalar.copy(_mkap(bufA2, 0, 128, 2 * R * blk + R, [(1, R)]),
                       _mkap(src, 0, 128, base + 3 * R, [(1, R)]))

    # ---- levels 2..6: block-range split DVE/Pool ----
    cur = bufA2
    dve_blocks = {2: 4, 3: 8, 4: 16, 5: 32, 6: 64}
    for i in range(2, 7):
        R = 1 << (12 - 2 * i)
        P = 1 << i
        nxt = d if i == 6 else (bufB if i % 2 == 0 else bufA2)
        bd = dve_blocks[i]
        for b0, nb, eng in ((0, bd, nc.vector), (bd, P - bd, nc.gpsimd)):
            if nb == 0:
                continue
            sdims = [(4 * R, nb), (1, R)] if R > 1 else [(4, nb)]
            ddims = [(2 * R, nb), (1, R)] if R > 1 else [(2, nb)]
            _acc3(eng,
                  _mkap(nxt, 0, 128, b0 * 2 * R, ddims),
                  _mkap(cur, 0, 128, b0 * 4 * R, sdims),
                  _mkap(cur, 0, 128, b0 * 4 * R + R, sdims),
                  _mkap(cur, 0, 128, b0 * 4 * R + 2 * R, sdims))
        cdims = [(4 * R, P), (1, R)] if R > 1 else [(4, P)]
        odims = [(2 * R, P), (1, R)] if R > 1 else [(2, P)]
        nc.scalar.copy(_mkap(nxt, 0, 128, R, odims),
                       _mkap(cur, 0, 128, 3 * R, cdims))
        cur = nxt


def _emit_body(nc, m, w, w_t, oall, pools):
    xpool, bpool, dpool, ppool = pools
    for g in range(4):
        x = xpool.tile([128, XPITCH], F32)
        for c in range(4):
            src = AP(m.ap().tensor, g * 128 * XPITCH + c * Q,
                     [[XPITCH, 128], [1, Q]])
            dst = AP(x.tensor, x.offset + c * Q, [[x.ap[0][0], 128], [1, Q]])
            nc.sync.dma_start(dst, src)
        if g == 0:
            nc.sync.dma_start(w_t[:], w.ap())

        bufB = bpool.tile([128, 4096], F32, tag="bufB")
        bufA2 = bpool.tile([128, 4096], F32, tag="bufA2")
        d = dpool.tile([128, 128], F32)
        _emit_levels(nc, x, bufB, bufA2, d)

        ps = ppool.tile([16, 128], F32)
        nc.tensor.matmul(ps[:], w_t[:], d[:], start=True, stop=True)
        nc.scalar.copy(_mkap(oall, 0, 16, g * 128, [(1, 128)]), ps[:])


def make_w() -> np.ndarray:
    w = np.zeros((128, 16), np.float32)
    for p in range(128):
        si = p >> 6
        r0, c0 = (p >> 5) & 1, (p >> 4) & 1
        r1, c1 = (p >> 3) & 1, (p >> 2) & 1
        r2, c2 = (p >> 1) & 1, p & 1
        mm = (r0 & c0) | ((r1 & c1) << 1) | ((r2 & c2) << 2)
        w[p][8 * si + mm] = 1.0
    return w


def build(iters: int = 1, timing_internal: bool = False) -> bass.Bass:
    """timing_internal: 'm' is Internal DRAM (no host upload), zero-filled
    on device before the loop - for loop-differencing timing only."""
    nc = bacc.Bacc("TRN2", target_bir_lowering=False, debug=False)
    mkind = "Internal" if timing_internal else "ExternalInput"
    m = nc.dram_tensor("m", [4, 128 * XPITCH], F32, kind=mkind)
    w = nc.dram_tensor("w", [128, 16], F32, kind="ExternalInput")
    out = nc.dram_tensor("out", [16, 512], F32, kind="ExternalOutput")
    from contextlib import ExitStack

    with tile.TileContext(nc) as tc:
        with ExitStack() as ctx:
            xpool = ctx.enter_context(tc.tile_pool(name="x", bufs=2))
            bpool = ctx.enter_context(tc.tile_pool(name="b", bufs=1))
            dpool = ctx.enter_context(tc.tile_pool(name="d", bufs=2))
            ppool = ctx.enter_context(tc.tile_pool(name="ps", bufs=2, space="PSUM"))
            wpool = ctx.enter_context(tc.tile_pool(name="w", bufs=1))
            w_t = wpool.tile([128, 16], F32)
            oall = wpool.tile([16, 512], F32, tag="oall")
            pools = (xpool, bpool, dpool, ppool)
            if timing_internal:
                zpool = ctx.enter_context(tc.tile_pool(name="z", bufs=1))
                z = zpool.tile([128, 2048], F32)
                nc.vector.memset(z[:], 0.0)
                for ch in range(32):
                    nc.sync.dma_start(
                        AP(m.ap().tensor,
                           (ch // 8) * 128 * XPITCH + (ch % 8) * 2048,
                           [[XPITCH, 128], [1, 2048]]),
                        AP(z.tensor, z.offset, [[z.ap[0][0], 128], [1, 2048]]))
            if iters == 1:
                _emit_body(nc, m, w, w_t, oall, pools)
            else:
                with tc.For_i(0, iters, 1):
                    _emit_body(nc, m, w, w_t, oall, pools)
            nc.sync.dma_start(out.ap(), oall[:])
    nc.compile()
    return nc


def _relayout(M: np.ndarray) -> np.ndarray:
    B = M.shape[0]
    Mv = M.reshape((B,) + (2,) * 20)
    r = {k: 1 + (9 - k) for k in range(10)}
    c = {k: 11 + (9 - k) for k in range(10)}
    perm = [0,
            r[0], c[0], r[1], c[1], r[2], c[2],
            r[9], c[9], r[8], c[8], r[7], c[7], r[6], c[6],
            r[5], c[5], r[4], c[4], r[3], c[3]]
    return np.ascontiguousarray(Mv.transpose(perm)).reshape(B, 64, XPITCH)


def _unscramble(res_cores: list[np.ndarray]) -> np.ndarray:
    outs = []
    for rk in res_cores:
        v = rk.reshape(2, 8, 4, 128)          # [si][m][g][e]
        v = v.transpose(2, 0, 3, 1)           # [g][si][e][m]
        outs.append(v.reshape(8, 1024))
    return np.concatenate(outs, axis=0)


def kernel(**inputs) -> np.ndarray:
    M = np.asarray(inputs["M"], dtype=np.float32)
    B = M.shape[0]
    X = _relayout(M.reshape(B, 1024, 1024))
    X = X.reshape(B // 2, 2 * 64 * XPITCH)
    nc = build(1)
    w = make_w()
    in_maps = [
        {"m": X[4 * k:4 * k + 4].reshape(4, 128 * XPITCH), "w": w}
        for k in range(NCORES)
    ]
    res = bass_utils.run_bass_kernel_spmd(nc, in_maps, core_ids=list(range(NCORES)))
    out = _unscramble([r["out"] for r in res.results])
    return out.reshape(B, 1024, 1, 1, 1)
